# revision 41
# baseline (speedup 1.0000x reference)
"""Causal single-head attention (B=8, T=2048, C=1024, H=64) on 8 trn2 NeuronCores.

Strategy (data-parallel over batch, one batch element per core):
  host: feed xT = x[b].T in fp16 (C becomes the on-chip contraction/partition
        dim and the dominant DMA halves), wqk = [Wq | Wk] fused projection
        weight, wv = [Wv | Wv].
  DMA:  block-0 x rides the scalar HWDGE ring interleaved with the weights
        (that ring kicks off ~3us before the sync ring), so the first proj
        matmul can start ~6us earlier; blocks 1-3 stream on the sync ring.
  proj: per 2-block group {b0,b1}: all b0 wqk chunk matmuls back-to-back
        (DMA-paced), then per chunk [b1 wqk matmul + a col-tiled wv pair]:
        the wv|wv fused stationary puts b0's v in psB partitions 0:64 and
        b1's in 64:128 concurrently (two 64-col matmuls in different PE
        column groups).
  qk:   psA (q rows 0:64, k rows 64:128) copied to SBUF with 3 casts:
        qk2a = aligned [128,512] copy, qk2b = the half-swapped duplicate.
        QK chunk parity alternates PE row halves: even chunks contract on
        partitions 64:128 (k from qk2a, q from qk2b), odd on 0:64.
  v:    one [128,512] cast psB->vtmp per group, then PE transposes per
        128-token chunk (ident halves match vtmp halves); ones column
        appended for the softmax denominator (65th PV output row).
  QK:   scores^T[s,q] per 128-wide s-chunk; causally trimmed moving width
        for the second diagonal pair of each block.
  exp:  one ACT op per chunk-pair [128,1024] -> pT fp16 with the 1/sqrt(C)
        scale via ACT's free affine pre-scale; the second diagonal pair
        uses two narrower ACT ops over exactly the causally-written cols.
  mask: triangular 128x128 multiply on diagonal chunks.
  PV:   out_aug^T[65, q] += v_aug-stationary @ pT-moving (causal widths).
        Final block drains psO stripes as each stripe's last PV lands;
        small keep-warm dummy matmuls stop the HAM clock gate from
        re-throttling the PE during the ACT-bound tail.
  out:  fp16 outT [65, T]; host divides rows 0:64 by row 64 and transposes.

fp16 everywhere on the PE (full rate, half DMA); accumulation in fp32 PSUM.
fp16 warm-up matmuls bridge the initial DMA wait so the HAM clock gate is at
K=8/8 when real work arrives.
"""

import numpy as np

import concourse.bass as bass
import concourse.mybir as mybir
import concourse.tile as tile
from concourse import bacc
from concourse.bass_utils import run_bass_kernel_spmd

B, T, C, H = 8, 2048, 1024, 64
TB = 512                 # q-block width
NBLK = T // TB           # 4 q-blocks
NC = C // 128            # 8 contraction chunks
NSC = T // 128           # 16 s-chunks
HA = H + 1               # v augmented with ones column
F32 = mybir.dt.float32
F16 = mybir.dt.float16

_compiled = {}


def build_nc():
    nc = bacc.Bacc("TRN2", target_bir_lowering=False, debug=False, num_devices=8)

    xT_d = nc.dram_tensor("xT", [C, T], F16, kind="ExternalInput").ap()
    wqk_d = nc.dram_tensor("wqk", [C, 128], F16, kind="ExternalInput").ap()
    wv_d = nc.dram_tensor("wv", [C, 128], F16, kind="ExternalInput").ap()
    # col 0:128 = causal upper-triangle mask, col 128 = ones
    tri_d = nc.dram_tensor("tri", [128, 129], F16, kind="ExternalInput").ap()
    outT_d = nc.dram_tensor("outT", [HA, T], F16, kind="ExternalOutput").ap()

    xT_r = xT_d.rearrange("(co ci) t -> ci co t", ci=128)
    wqk_r = wqk_d.rearrange("(co ci) m -> ci co m", ci=128)
    wv_r = wv_d.rearrange("(co ci) m -> ci co m", ci=128)

    with tile.TileContext(nc) as tc:
        with (
            tc.tile_pool(name="const", bufs=1) as cpool,
            tc.tile_pool(name="persist", bufs=1) as ppool,
            tc.tile_pool(name="xin", bufs=10) as xpool,
            tc.tile_pool(name="ptile", bufs=6) as pt_pool,
            tc.tile_pool(name="vtmp", bufs=2) as vtmp_pool,
            tc.tile_pool(name="outsb", bufs=2) as out_pool,
            tc.tile_pool(name="psA", bufs=2, space="PSUM") as psA_pool,
            tc.tile_pool(name="psB", bufs=1, space="PSUM") as psB_pool,
            tc.tile_pool(name="psQK", bufs=2, space="PSUM") as psQK_pool,
            tc.tile_pool(name="psO", bufs=1, space="PSUM") as psO_pool,
        ):
            wqk_s = cpool.tile([128, NC, 128], F16)
            wv_s = cpool.tile([128, NC, 128], F16)
            tri_full = cpool.tile([128, 129], F16)
            tri_s = tri_full[:, 0:128]
            ones_s = tri_full[:, 128:129]
            ident = cpool.tile([128, 64], F16)
            warm = cpool.tile([128, 512], F16)

            # weights as few large transfers on the scalar HWDGE ring (each
            # dma_start costs ~0.5us of serialized descriptor generation, and
            # the scalar queue must stay clear for the exps); chunk 0 first so
            # the first matmul's weights land ASAP
            nc.scalar.dma_start(wqk_s[:, 0:1, :], wqk_r[:, 0:1, :])
            nc.scalar.dma_start(wqk_s[:, 1:NC, :], wqk_r[:, 1:NC, :])
            nc.scalar.dma_start(wv_s[:], wv_r[:])
            nc.scalar.dma_start(tri_full[:], tri_d[:])
            # x streams in half-block transfers (each dma_start costs ~0.8us
            # of serialized sequencer descriptor generation, so chunked DMAs
            # would cap the stream at ~155 GB/s).  Block 0 rides the scalar
            # ring, which reliably starts ~3us before the sync ring and gates
            # the whole exp stream; blocks 1-3 stream on the sync ring in
            # parallel.
            x_c = [[None] * NC for _ in range(NBLK)]
            for i in range(NBLK):
                q0 = i * TB
                ring = nc.scalar if i == 0 else nc.sync
                for h in range(2):
                    xh = xpool.tile([128, NC // 2, TB], F16)
                    ring.dma_start(
                        xh[:], xT_r[:, h * (NC // 2) : (h + 1) * (NC // 2),
                                    q0 : q0 + TB]
                    )
                    for c4 in range(NC // 2):
                        x_c[i][h * (NC // 2) + c4] = xh[:, c4, :]

            # PE warm-up: fp16 matmuls with no DMA deps bridge the initial
            # DMA wait and push the HAM clock gate toward K=8/8
            nc.gpsimd.memset(warm[:], 0.0)
            for w in range(8):
                ps_warm = psQK_pool.tile([128, 512], F32, tag="psQK")
                nc.tensor.matmul(ps_warm[:], warm[:, 0:128], warm[:],
                                 start=True, stop=True)
            for h2 in range(2):
                sl = ident[h2 * 64 : (h2 + 1) * 64, :]
                nc.gpsimd.memset(sl, 0.0)
                nc.gpsimd.affine_select(
                    out=sl,
                    in_=sl,
                    compare_op=mybir.AluOpType.not_equal,
                    fill=1.0,
                    base=0,
                    pattern=[[-1, 64]],
                    channel_multiplier=1,
                )

            # qk2a: q rows 0:64, k rows 64:128 (aligned copy of psA)
            # qk2b: k rows 0:64, q rows 64:128 (half-swapped duplicate)
            qk2a = ppool.tile([128, T], F16)
            qk2b = ppool.tile([128, T], F16)
            v_s = ppool.tile([128, NSC * HA], F16)

            psA_t = [None] * NBLK
            psB_t = [None] * (NBLK // 2)
            vtmp_t = [None] * (NBLK // 2)

            def wqk_part(b, c0, c1):
                """q/k proj matmuls for block b, chunks [c0, c1); allocates
                psA at c0==0 and emits the casts at c1==NC."""
                if c0 == 0:
                    psA_t[b] = psA_pool.tile([128, TB], F32, name="psA")
                psA = psA_t[b]
                for c in range(c0, c1):
                    nc.tensor.matmul(
                        psA[:], wqk_s[:, c, :], x_c[b][c][:],
                        start=(c == 0), stop=(c == NC - 1),
                    )
                if c1 == NC:
                    qk_casts(b)
                    if b < 2:
                        # PE idles for the cast latency before the first QKs;
                        # dangling LDWEIGHTS keep the HAM clock gate from
                        # re-throttling during the ramp
                        for _ in range(10):
                            nc.tensor.ldweights(warm[:, 0:128])

            def wv_part(gi, c0, c1):
                """col-tiled wv pairs: b0 -> psB[0:64] (PE col group 0-1),
                b1 -> psB[64:128] (col group 2-3) run concurrently."""
                b0, b1 = 2 * gi, 2 * gi + 1
                if c0 == 0:
                    psB_t[gi] = psB_pool.tile([128, TB], F32, name="psB")
                psB = psB_t[gi]
                for c in range(c0, c1):
                    nc.tensor.matmul(
                        psB[0:64], wv_s[:, c, 0:64], x_c[b0][c][:],
                        start=(c == 0), stop=(c == NC - 1),
                    )
                    nc.tensor.matmul(
                        psB[64:128], wv_s[:, c, 64:128], x_c[b1][c][:],
                        start=(c == 0), stop=(c == NC - 1),
                    )

            def qk_casts(i):
                q0 = i * TB
                psA = psA_t[i]
                nc.vector.tensor_copy(qk2a[:, q0 : q0 + TB], psA[0:128, :])
                if i < 2:
                    # scalar engine is idle before the exp stream starts:
                    # run the q-dup cast there in parallel with the DVE pair
                    nc.scalar.copy(qk2b[64:128, q0 : q0 + TB], psA[0:64, :])
                else:
                    nc.vector.tensor_copy(qk2b[64:128, q0 : q0 + TB], psA[0:64, :])
                nc.vector.tensor_copy(qk2b[0:64, q0 : q0 + TB], psA[64:128, :])

            def vt_part(gi, db):
                """PE transposes for one block of the group; db==0 also does
                the psB->vtmp cast covering both blocks."""
                b0 = 2 * gi
                if db == 0:
                    vtmp_t[gi] = vtmp_pool.tile([128, TB], F16, name="vtmp")
                    nc.vector.tensor_copy(vtmp_t[gi][:], psB_t[gi][:])
                vtmp = vtmp_t[gi]
                r = slice(db * 64, db * 64 + 64)
                for j4 in range(TB // 128):
                    sj = (TB // 128) * (b0 + db) + j4
                    ps_vt = psQK_pool.tile([128, 64], F16, tag="psQK")
                    nc.tensor.transpose(
                        ps_vt[:],
                        vtmp[r, j4 * 128 : (j4 + 1) * 128],
                        ident[r, :],
                    )
                    nc.vector.tensor_copy(v_s[:, sj * HA : sj * HA + H], ps_vt[:])
                    # ones column via gpsimd (idle engine) to keep DVE free
                    nc.gpsimd.memset(
                        v_s[:, sj * HA + H : sj * HA + HA], 1.0
                    )

            def qk_mm(j, q0, lo, dst):
                """scores^T for s-chunk j against q cols [q0+lo, q0+TB);
                chunk parity picks the PE row half."""
                if j % 2 == 0:
                    stat, mov = qk2a[64:128, :], qk2b[64:128, :]
                else:
                    stat, mov = qk2b[0:64, :], qk2a[0:64, :]
                nc.tensor.matmul(
                    dst,
                    stat[:, j * 128 : (j + 1) * 128],
                    mov[:, q0 + lo : q0 + TB],
                    start=True, stop=True,
                )

            scale = float(1.0 / np.sqrt(C))

            # pair list in stream order: (block, pair-in-block)
            pairs = [(i, g) for i in range(NBLK)
                     for g in range((TB // 128) * (i + 1) // 2)]
            pT_t = {}
            psO_t = [None] * NBLK
            out_sb_last = [None]

            def pair_geom(i, g):
                q0 = i * TB
                js = [2 * g, 2 * g + 1]
                ds = [j * 128 - q0 for j in js]
                los = [max(dd, 0) for dd in ds]
                diagB = los[0] > 0  # second diagonal pair of the block
                return q0, js, ds, los, diagB

            def scores(i, g):
                """QK + exp (+nothing that needs v): produces pT_t[(i,g)]."""
                q0, js, ds, los, diagB = pair_geom(i, g)
                psQK = psQK_pool.tile([128, 1024], F32, tag="psQK")
                for h2 in range(2):
                    lo = los[h2] if diagB else 0
                    qk_mm(js[h2], q0, lo,
                          psQK[:, h2 * 512 + lo : (h2 + 1) * 512])
                pT = pt_pool.tile([128, 1024], F16)
                pT_t[(i, g)] = pT
                if diagB:
                    # two ACT ops over exactly the causally-written cols
                    for h2 in range(2):
                        lo = los[h2]
                        nc.scalar.activation(
                            pT[:, h2 * 512 + lo : (h2 + 1) * 512],
                            psQK[:, h2 * 512 + lo : (h2 + 1) * 512],
                            mybir.ActivationFunctionType.Exp,
                            scale=scale,
                        )
                else:
                    nc.scalar.activation(
                        pT[:], psQK[:], mybir.ActivationFunctionType.Exp,
                        scale=scale,
                    )
                for h2 in range(2):
                    d = ds[h2]
                    if d >= 0:  # diagonal chunk: triangular causal mask
                        pj = pT[:, h2 * 512 : (h2 + 1) * 512]
                        nc.vector.tensor_mul(
                            pj[:, d : d + 128], pj[:, d : d + 128], tri_s[:]
                        )

            def pv(i, g):
                q0, js, ds, los, _ = pair_geom(i, g)
                nsc_i = (TB // 128) * (i + 1)
                if g == 0:
                    psO_t[i] = psO_pool.tile([HA, TB], F32, name="psO")
                    if i == NBLK - 1:
                        out_sb_last[0] = out_pool.tile([HA, TB], F16,
                                                       name="out_sb_last")
                psO = psO_t[i]
                pT = pT_t.pop((i, g))
                if i >= 2:
                    # keep-warm: the late blocks are ACT-bound and PE
                    # micro-idles re-throttle the HAM clock gate; dangling
                    # LDWEIGHTS fill the exp-wait stall and are overwritten
                    # by the next matmul's own weight load
                    nc.tensor.ldweights(warm[:, 0:128])
                    nc.tensor.ldweights(warm[:, 0:128])
                for h2 in range(2):
                    j, lo = js[h2], los[h2]
                    pj = pT[:, h2 * 512 : (h2 + 1) * 512]
                    nc.tensor.matmul(
                        psO[:, lo:TB],
                        v_s[:, j * HA : (j + 1) * HA],
                        pj[:, lo:TB],
                        start=(j == 0), stop=(j == nsc_i - 1),
                    )
                    if i == NBLK - 1 and j >= nsc_i - 4:
                        # drain each psO stripe as its last PV lands
                        p = j - (nsc_i - 4)
                        sl = slice(p * 128, (p + 1) * 128)
                        nc.vector.tensor_copy(out_sb_last[0][:, sl], psO[:, sl])
                        nc.gpsimd.dma_start(
                            outT_d[:, q0 + p * 128 : q0 + (p + 1) * 128],
                            out_sb_last[0][:, sl],
                        )
                if i < NBLK - 1 and g == nsc_i // 2 - 1:
                    out_sb = out_pool.tile([HA, TB], F16)
                    nc.vector.tensor_copy(out_sb[:], psO[:])
                    nc.gpsimd.dma_start(outT_d[:, q0 : q0 + TB], out_sb[:])

            # Hand-scheduled emission: the Tile static scheduler re-sorts by
            # its own cost model, so each group is pinned to an increasing
            # tile_wait_until slot (a pure scheduler-time gate, no runtime
            # waits) to force the engine-queue order.  Bulk proj/v work is
            # sliced between scores pairs so the exp stream stays dense; the
            # scores stream runs ahead of the pv stream by at most 6 pairs
            # (the pT pool depth).
            S = lambda k: scores(*pairs[k])
            P = lambda k: pv(*pairs[k])
            groups = [
                lambda: wqk_part(0, 0, NC),     # block 0 proj + casts
                lambda: wqk_part(1, 0, 4),      # fills the casts(0) latency
                lambda: (S(0), S(1)),           # attn0 scores
                lambda: wqk_part(1, 4, NC),     # block 1 proj + casts
                lambda: S(2),                   # (1,0)
                lambda: wv_part(0, 0, 4),
                lambda: S(3),                   # (1,1)
                lambda: wv_part(0, 4, NC),
                lambda: S(4),                   # (1,2)
                lambda: vt_part(0, 0),
                lambda: S(5),                   # (1,3)
                lambda: vt_part(0, 1),
                lambda: (P(0), P(1)),           # attn0 pv
                lambda: wqk_part(2, 0, NC),     # block 2 proj + casts
                lambda: P(2),                   # (1,0)
                lambda: S(6),                   # (2,0)
                lambda: P(3),
                lambda: wqk_part(3, 0, 4),
                lambda: S(7),                   # (2,1)
                lambda: wqk_part(3, 4, NC),     # block 3 proj + casts
                lambda: P(4),
                lambda: S(8),                   # (2,2)
                lambda: wv_part(1, 0, 4),
                lambda: P(5),
                lambda: S(9),                   # (2,3)
                lambda: wv_part(1, 4, NC),
                lambda: P(6),                   # (2,0)
                lambda: vt_part(1, 0),
                lambda: S(10),                  # (2,4)
                lambda: P(7),
                lambda: vt_part(1, 1),
                lambda: S(11),                  # (2,5)
                lambda: P(8),
                lambda: S(12),                  # (3,0)
                lambda: P(9),
                lambda: S(13),
                lambda: P(10),
                lambda: S(14),
                lambda: P(11),
                lambda: S(15),
                lambda: P(12),
                lambda: S(16),
                lambda: P(13),
                lambda: S(17),
                lambda: P(14),
                lambda: S(18),
                lambda: P(15),
                lambda: S(19),
                lambda: (P(16), P(17), P(18), P(19)),
            ]
            for group in groups:
                group()

    nc.compile()
    return nc


def _get_nc():
    if "nc" not in _compiled:
        _compiled["nc"] = build_nc()
    return _compiled["nc"]


def make_in_maps(x, Wk, Wq, Wv):
    x = np.asarray(x, dtype=np.float32)
    Wk = np.asarray(Wk, dtype=np.float32)
    Wq = np.asarray(Wq, dtype=np.float32)
    Wv = np.asarray(Wv, dtype=np.float32)
    # raw Wq (no 1/sqrt(C) here — that scale rides the exp's affine pre-scale)
    wqk = np.concatenate([Wq, Wk], axis=1).astype(np.float16)  # [C, 128]
    wvd = np.concatenate([Wv, Wv], axis=1).astype(np.float16)  # [C, 128] dup
    tri = np.ones((128, 129), dtype=np.float16)
    tri[:, 0:128] = np.triu(np.ones((128, 128), dtype=np.float16))
    in_maps = []
    for b in range(B):
        in_maps.append(
            {
                "xT": np.ascontiguousarray(x[b].T.astype(np.float16)),
                "wqk": wqk,
                "wv": wvd,
                "tri": tri,
            }
        )
    return in_maps


def postprocess(results):
    outs = []
    for b in range(B):
        outT = results[b]["outT"].astype(np.float32)  # [65, T]
        out = (outT[:H] / outT[H : H + 1]).T  # [T, H]
        outs.append(out)
    return np.stack(outs).astype(np.float32)


def run(x, Wk, Wq, Wv, trace=False, **kw):
    nc = _get_nc()
    in_maps = make_in_maps(x, Wk, Wq, Wv)
    res = run_bass_kernel_spmd(
        nc, in_maps, core_ids=list(range(B)), trace=trace, **kw
    )
    return postprocess(res.results), res


def kernel(x, Wk, Wq, Wv):
    out, _ = run(x, Wk, Wq, Wv, trace=False)
    return out


# revision 43
# speedup vs baseline: 1.0835x; 1.0835x over previous
"""Causal single-head attention (B=8, T=2048, C=1024, H=64) on 8 trn2 NeuronCores.

Strategy (data-parallel over batch, one batch element per core):
  host: feed xT = x[b].T in fp16 (C becomes the on-chip contraction/partition
        dim and the dominant DMA halves), wqk = [Wq | Wk] fused projection
        weight, wv = [Wv | Wv].
  DMA:  block-0 x rides the scalar HWDGE ring interleaved with the weights
        (that ring kicks off ~3us before the sync ring), so the first proj
        matmul can start ~6us earlier; blocks 1-3 stream on the sync ring.
  proj: per 2-block group {b0,b1}: all b0 wqk chunk matmuls back-to-back
        (DMA-paced), then per chunk [b1 wqk matmul + a col-tiled wv pair]:
        the wv|wv fused stationary puts b0's v in psB partitions 0:64 and
        b1's in 64:128 concurrently (two 64-col matmuls in different PE
        column groups).
  qk:   psA (q rows 0:64, k rows 64:128) copied to SBUF with 3 casts:
        qk2a = aligned [128,512] copy, qk2b = the half-swapped duplicate.
        QK chunk parity alternates PE row halves: even chunks contract on
        partitions 64:128 (k from qk2a, q from qk2b), odd on 0:64.
  v:    one [128,512] cast psB->vtmp per group, then PE transposes per
        128-token chunk (ident halves match vtmp halves); ones column
        appended for the softmax denominator (65th PV output row).
  QK:   scores^T[s,q] per 128-wide s-chunk; causally trimmed moving width
        for the second diagonal pair of each block.
  exp:  one ACT op per chunk-pair [128,1024] -> pT fp16 with the 1/sqrt(C)
        scale via ACT's free affine pre-scale; the second diagonal pair
        uses two narrower ACT ops over exactly the causally-written cols.
  mask: triangular 128x128 multiply on diagonal chunks.
  PV:   out_aug^T[65, q] += v_aug-stationary @ pT-moving (causal widths).
        Final block drains psO stripes as each stripe's last PV lands;
        small keep-warm dummy matmuls stop the HAM clock gate from
        re-throttling the PE during the ACT-bound tail.
  out:  fp16 outT [65, T]; host divides rows 0:64 by row 64 and transposes.

fp16 everywhere on the PE (full rate, half DMA); accumulation in fp32 PSUM.
fp16 warm-up matmuls bridge the initial DMA wait so the HAM clock gate is at
K=8/8 when real work arrives.
"""

import numpy as np

import concourse.bass as bass
import concourse.mybir as mybir
import concourse.tile as tile
from concourse import bacc
from concourse.bass_utils import run_bass_kernel_spmd

B, T, C, H = 8, 2048, 1024, 64
TB = 512                 # q-block width
NBLK = T // TB           # 4 q-blocks
NC = C // 128            # 8 contraction chunks
NSC = T // 128           # 16 s-chunks
HA = H + 1               # v augmented with ones column
F32 = mybir.dt.float32
F16 = mybir.dt.float16

_compiled = {}


def build_nc():
    nc = bacc.Bacc("TRN2", target_bir_lowering=False, debug=False, num_devices=8)

    xT_d = nc.dram_tensor("xT", [C, T], F16, kind="ExternalInput").ap()
    wqk_d = nc.dram_tensor("wqk", [C, 128], F16, kind="ExternalInput").ap()
    wv_d = nc.dram_tensor("wv", [C, 128], F16, kind="ExternalInput").ap()
    # col 0:128 = causal upper-triangle mask, col 128 = ones
    tri_d = nc.dram_tensor("tri", [128, 129], F16, kind="ExternalInput").ap()
    outT_d = nc.dram_tensor("outT", [HA, T], F16, kind="ExternalOutput").ap()

    xT_r = xT_d.rearrange("(co ci) t -> ci co t", ci=128)
    wqk_r = wqk_d.rearrange("(co ci) m -> ci co m", ci=128)
    wv_r = wv_d.rearrange("(co ci) m -> ci co m", ci=128)

    with tile.TileContext(nc) as tc:
        with (
            tc.tile_pool(name="const", bufs=1) as cpool,
            tc.tile_pool(name="persist", bufs=1) as ppool,
            tc.tile_pool(name="xin", bufs=10) as xpool,
            tc.tile_pool(name="ptile", bufs=6) as pt_pool,
            tc.tile_pool(name="vtmp", bufs=2) as vtmp_pool,
            tc.tile_pool(name="outsb", bufs=2) as out_pool,
            tc.tile_pool(name="psA", bufs=2, space="PSUM") as psA_pool,
            tc.tile_pool(name="psB", bufs=1, space="PSUM") as psB_pool,
            tc.tile_pool(name="psQK", bufs=2, space="PSUM") as psQK_pool,
            tc.tile_pool(name="psO", bufs=1, space="PSUM") as psO_pool,
        ):
            wqk_s = cpool.tile([128, NC, 128], F16)
            wv_s = cpool.tile([128, NC, 128], F16)
            tri_full = cpool.tile([128, 129], F16)
            tri_s = tri_full[:, 0:128]
            ones_s = tri_full[:, 128:129]
            ident = cpool.tile([128, 64], F16)
            warm = cpool.tile([128, 512], F16)

            # weights as few large transfers on the scalar HWDGE ring (each
            # dma_start costs ~0.5us of serialized descriptor generation, and
            # the scalar queue must stay clear for the exps); chunk 0 first so
            # the first matmul's weights land ASAP
            nc.scalar.dma_start(wqk_s[:, 0:1, :], wqk_r[:, 0:1, :])
            nc.scalar.dma_start(wqk_s[:, 1:NC, :], wqk_r[:, 1:NC, :])
            nc.scalar.dma_start(wv_s[:], wv_r[:])
            nc.scalar.dma_start(tri_full[:], tri_d[:])
            # x streams in half-block transfers (each dma_start costs ~0.8us
            # of serialized sequencer descriptor generation, so chunked DMAs
            # would cap the stream at ~155 GB/s).  Block 0 rides the scalar
            # ring, which reliably starts ~3us before the sync ring and gates
            # the whole exp stream; blocks 1-3 stream on the sync ring in
            # parallel.
            x_c = [[None] * NC for _ in range(NBLK)]
            for i in range(NBLK):
                q0 = i * TB
                ring = nc.sync
                for h in range(2):
                    xh = xpool.tile([128, NC // 2, TB], F16)
                    ring.dma_start(
                        xh[:], xT_r[:, h * (NC // 2) : (h + 1) * (NC // 2),
                                    q0 : q0 + TB]
                    )
                    for c4 in range(NC // 2):
                        x_c[i][h * (NC // 2) + c4] = xh[:, c4, :]

            # PE warm-up: fp16 matmuls with no DMA deps bridge the initial
            # DMA wait and push the HAM clock gate toward K=8/8
            nc.gpsimd.memset(warm[:], 0.0)
            for w in range(8):
                ps_warm = psQK_pool.tile([128, 512], F32, tag="psQK")
                nc.tensor.matmul(ps_warm[:], warm[:, 0:128], warm[:],
                                 start=True, stop=True)
            for h2 in range(2):
                sl = ident[h2 * 64 : (h2 + 1) * 64, :]
                nc.gpsimd.memset(sl, 0.0)
                nc.gpsimd.affine_select(
                    out=sl,
                    in_=sl,
                    compare_op=mybir.AluOpType.not_equal,
                    fill=1.0,
                    base=0,
                    pattern=[[-1, 64]],
                    channel_multiplier=1,
                )

            # qk2a: q rows 0:64, k rows 64:128 (aligned copy of psA)
            # qk2b: k rows 0:64, q rows 64:128 (half-swapped duplicate)
            qk2a = ppool.tile([128, T], F16)
            qk2b = ppool.tile([128, T], F16)
            v_s = ppool.tile([128, NSC * HA], F16)

            psA_t = [None] * NBLK
            psB_t = [None] * (NBLK // 2)
            vtmp_t = [None] * (NBLK // 2)

            def wqk_part(b, c0, c1):
                """q/k proj matmuls for block b, chunks [c0, c1); allocates
                psA at c0==0 and emits the casts at c1==NC."""
                if c0 == 0:
                    psA_t[b] = psA_pool.tile([128, TB], F32, name="psA")
                psA = psA_t[b]
                for c in range(c0, c1):
                    nc.tensor.matmul(
                        psA[:], wqk_s[:, c, :], x_c[b][c][:],
                        start=(c == 0), stop=(c == NC - 1),
                    )
                if c1 == NC:
                    qk_casts(b)
                    if b < 2:
                        # PE idles for the cast latency before the first QKs;
                        # dangling LDWEIGHTS keep the HAM clock gate from
                        # re-throttling during the ramp
                        for _ in range(10):
                            nc.tensor.ldweights(warm[:, 0:128])

            def wv_part(gi, c0, c1):
                """col-tiled wv pairs: b0 -> psB[0:64] (PE col group 0-1),
                b1 -> psB[64:128] (col group 2-3) run concurrently."""
                b0, b1 = 2 * gi, 2 * gi + 1
                if c0 == 0:
                    psB_t[gi] = psB_pool.tile([128, TB], F32, name="psB")
                psB = psB_t[gi]
                for c in range(c0, c1):
                    nc.tensor.matmul(
                        psB[0:64], wv_s[:, c, 0:64], x_c[b0][c][:],
                        start=(c == 0), stop=(c == NC - 1),
                    )
                    nc.tensor.matmul(
                        psB[64:128], wv_s[:, c, 64:128], x_c[b1][c][:],
                        start=(c == 0), stop=(c == NC - 1),
                    )

            def qk_casts(i):
                q0 = i * TB
                psA = psA_t[i]
                nc.vector.tensor_copy(qk2a[:, q0 : q0 + TB], psA[0:128, :])
                if i < 2:
                    # scalar engine is idle before the exp stream starts:
                    # run the q-dup cast there in parallel with the DVE pair
                    nc.scalar.copy(qk2b[64:128, q0 : q0 + TB], psA[0:64, :])
                else:
                    nc.vector.tensor_copy(qk2b[64:128, q0 : q0 + TB], psA[0:64, :])
                nc.vector.tensor_copy(qk2b[0:64, q0 : q0 + TB], psA[64:128, :])

            def vt_part(gi, db):
                """PE transposes for one block of the group; db==0 also does
                the psB->vtmp cast covering both blocks."""
                b0 = 2 * gi
                if db == 0:
                    vtmp_t[gi] = vtmp_pool.tile([128, TB], F16, name="vtmp")
                    nc.vector.tensor_copy(vtmp_t[gi][:], psB_t[gi][:])
                vtmp = vtmp_t[gi]
                r = slice(db * 64, db * 64 + 64)
                for j4 in range(TB // 128):
                    sj = (TB // 128) * (b0 + db) + j4
                    ps_vt = psQK_pool.tile([128, 64], F16, tag="psQK")
                    nc.tensor.transpose(
                        ps_vt[:],
                        vtmp[r, j4 * 128 : (j4 + 1) * 128],
                        ident[r, :],
                    )
                    nc.vector.tensor_copy(v_s[:, sj * HA : sj * HA + H], ps_vt[:])
                    # ones column via gpsimd (idle engine) to keep DVE free
                    nc.gpsimd.memset(
                        v_s[:, sj * HA + H : sj * HA + HA], 1.0
                    )

            def qk_mm(j, q0, lo, dst):
                """scores^T for s-chunk j against q cols [q0+lo, q0+TB);
                chunk parity picks the PE row half."""
                if j % 2 == 0:
                    stat, mov = qk2a[64:128, :], qk2b[64:128, :]
                else:
                    stat, mov = qk2b[0:64, :], qk2a[0:64, :]
                nc.tensor.matmul(
                    dst,
                    stat[:, j * 128 : (j + 1) * 128],
                    mov[:, q0 + lo : q0 + TB],
                    start=True, stop=True,
                )

            scale = float(1.0 / np.sqrt(C))

            # pair list in stream order: (block, pair-in-block)
            pairs = [(i, g) for i in range(NBLK)
                     for g in range((TB // 128) * (i + 1) // 2)]
            pT_t = {}
            psO_t = [None] * NBLK
            out_sb_last = [None]

            def pair_geom(i, g):
                q0 = i * TB
                js = [2 * g, 2 * g + 1]
                ds = [j * 128 - q0 for j in js]
                los = [max(dd, 0) for dd in ds]
                diagB = los[0] > 0  # second diagonal pair of the block
                return q0, js, ds, los, diagB

            def scores(i, g):
                """QK + exp (+nothing that needs v): produces pT_t[(i,g)]."""
                q0, js, ds, los, diagB = pair_geom(i, g)
                psQK = psQK_pool.tile([128, 1024], F32, tag="psQK")
                for h2 in range(2):
                    lo = los[h2] if diagB else 0
                    qk_mm(js[h2], q0, lo,
                          psQK[:, h2 * 512 + lo : (h2 + 1) * 512])
                pT = pt_pool.tile([128, 1024], F16)
                pT_t[(i, g)] = pT
                if diagB:
                    # two ACT ops over exactly the causally-written cols
                    for h2 in range(2):
                        lo = los[h2]
                        nc.scalar.activation(
                            pT[:, h2 * 512 + lo : (h2 + 1) * 512],
                            psQK[:, h2 * 512 + lo : (h2 + 1) * 512],
                            mybir.ActivationFunctionType.Exp,
                            scale=scale,
                        )
                else:
                    nc.scalar.activation(
                        pT[:], psQK[:], mybir.ActivationFunctionType.Exp,
                        scale=scale,
                    )
                for h2 in range(2):
                    d = ds[h2]
                    if d >= 0:  # diagonal chunk: triangular causal mask
                        pj = pT[:, h2 * 512 : (h2 + 1) * 512]
                        nc.vector.tensor_mul(
                            pj[:, d : d + 128], pj[:, d : d + 128], tri_s[:]
                        )

            def pv(i, g):
                q0, js, ds, los, _ = pair_geom(i, g)
                nsc_i = (TB // 128) * (i + 1)
                if g == 0:
                    psO_t[i] = psO_pool.tile([HA, TB], F32, name="psO")
                    if i == NBLK - 1:
                        out_sb_last[0] = out_pool.tile([HA, TB], F16,
                                                       name="out_sb_last")
                psO = psO_t[i]
                pT = pT_t.pop((i, g))
                if i >= 2:
                    # keep-warm: the late blocks are ACT-bound and PE
                    # micro-idles re-throttle the HAM clock gate; a dangling
                    # LDWEIGHTS fills the exp-wait stall and is overwritten
                    # by the next matmul's own weight load
                    nc.tensor.ldweights(warm[:, 0:128])
                for h2 in range(2):
                    j, lo = js[h2], los[h2]
                    pj = pT[:, h2 * 512 : (h2 + 1) * 512]
                    nc.tensor.matmul(
                        psO[:, lo:TB],
                        v_s[:, j * HA : (j + 1) * HA],
                        pj[:, lo:TB],
                        start=(j == 0), stop=(j == nsc_i - 1),
                    )
                    if i == NBLK - 1 and j >= nsc_i - 4:
                        # drain each psO stripe as its last PV lands
                        p = j - (nsc_i - 4)
                        sl = slice(p * 128, (p + 1) * 128)
                        nc.vector.tensor_copy(out_sb_last[0][:, sl], psO[:, sl])
                        nc.gpsimd.dma_start(
                            outT_d[:, q0 + p * 128 : q0 + (p + 1) * 128],
                            out_sb_last[0][:, sl],
                        )
                if i < NBLK - 1 and g == nsc_i // 2 - 1:
                    out_sb = out_pool.tile([HA, TB], F16)
                    nc.vector.tensor_copy(out_sb[:], psO[:])
                    nc.gpsimd.dma_start(outT_d[:, q0 : q0 + TB], out_sb[:])

            # Hand-scheduled emission: the Tile static scheduler re-sorts by
            # its own cost model, so each group is pinned to an increasing
            # tile_wait_until slot (a pure scheduler-time gate, no runtime
            # waits) to force the engine-queue order.  Bulk proj/v work is
            # sliced between scores pairs so the exp stream stays dense; the
            # scores stream runs ahead of the pv stream by at most 6 pairs
            # (the pT pool depth).
            S = lambda k: scores(*pairs[k])
            P = lambda k: pv(*pairs[k])
            groups = [
                lambda: wqk_part(0, 0, NC),     # block 0 proj + casts
                lambda: wqk_part(1, 0, 4),      # fills the casts(0) latency
                lambda: (S(0), S(1)),           # attn0 scores
                lambda: wqk_part(1, 4, NC),     # block 1 proj + casts
                lambda: S(2),                   # (1,0)
                lambda: wv_part(0, 0, 4),
                lambda: S(3),                   # (1,1)
                lambda: wv_part(0, 4, NC),
                lambda: S(4),                   # (1,2)
                lambda: vt_part(0, 0),
                lambda: S(5),                   # (1,3)
                lambda: vt_part(0, 1),
                lambda: (P(0), P(1)),           # attn0 pv
                lambda: wqk_part(2, 0, NC),     # block 2 proj + casts
                lambda: P(2),                   # (1,0)
                lambda: S(6),                   # (2,0)
                lambda: P(3),
                lambda: wqk_part(3, 0, 4),
                lambda: S(7),                   # (2,1)
                lambda: wqk_part(3, 4, NC),     # block 3 proj + casts
                lambda: P(4),
                lambda: S(8),                   # (2,2)
                lambda: wv_part(1, 0, 4),
                lambda: P(5),
                lambda: S(9),                   # (2,3)
                lambda: wv_part(1, 4, NC),
                lambda: P(6),                   # (2,0)
                lambda: vt_part(1, 0),
                lambda: S(10),                  # (2,4)
                lambda: P(7),
                lambda: vt_part(1, 1),
                lambda: S(11),                  # (2,5)
                lambda: P(8),
                lambda: S(12),                  # (3,0)
                lambda: P(9),
                lambda: S(13),
                lambda: P(10),
                lambda: S(14),
                lambda: P(11),
                lambda: S(15),
                lambda: P(12),
                lambda: S(16),
                lambda: P(13),
                lambda: S(17),
                lambda: P(14),
                lambda: S(18),
                lambda: P(15),
                lambda: S(19),
                lambda: (P(16), P(17), P(18), P(19)),
            ]
            for group in groups:
                group()

    nc.compile()
    return nc


def _get_nc():
    if "nc" not in _compiled:
        _compiled["nc"] = build_nc()
    return _compiled["nc"]


def make_in_maps(x, Wk, Wq, Wv):
    x = np.asarray(x, dtype=np.float32)
    Wk = np.asarray(Wk, dtype=np.float32)
    Wq = np.asarray(Wq, dtype=np.float32)
    Wv = np.asarray(Wv, dtype=np.float32)
    # raw Wq (no 1/sqrt(C) here — that scale rides the exp's affine pre-scale)
    wqk = np.concatenate([Wq, Wk], axis=1).astype(np.float16)  # [C, 128]
    wvd = np.concatenate([Wv, Wv], axis=1).astype(np.float16)  # [C, 128] dup
    tri = np.ones((128, 129), dtype=np.float16)
    tri[:, 0:128] = np.triu(np.ones((128, 128), dtype=np.float16))
    in_maps = []
    for b in range(B):
        in_maps.append(
            {
                "xT": np.ascontiguousarray(x[b].T.astype(np.float16)),
                "wqk": wqk,
                "wv": wvd,
                "tri": tri,
            }
        )
    return in_maps


def postprocess(results):
    outs = []
    for b in range(B):
        outT = results[b]["outT"].astype(np.float32)  # [65, T]
        out = (outT[:H] / outT[H : H + 1]).T  # [T, H]
        outs.append(out)
    return np.stack(outs).astype(np.float32)


def run(x, Wk, Wq, Wv, trace=False, **kw):
    nc = _get_nc()
    in_maps = make_in_maps(x, Wk, Wq, Wv)
    res = run_bass_kernel_spmd(
        nc, in_maps, core_ids=list(range(B)), trace=trace, **kw
    )
    return postprocess(res.results), res


def kernel(x, Wk, Wq, Wv):
    out, _ = run(x, Wk, Wq, Wv, trace=False)
    return out


# revision 44
# speedup vs baseline: 1.0962x; 1.0117x over previous
"""Causal single-head attention (B=8, T=2048, C=1024, H=64) on 8 trn2 NeuronCores.

Strategy (data-parallel over batch, one batch element per core):
  host: feed xT = x[b].T in fp16 (C becomes the on-chip contraction/partition
        dim and the dominant DMA halves), wqk = [Wq | Wk] fused projection
        weight, wv = [Wv | Wv].
  DMA:  block-0 x rides the scalar HWDGE ring interleaved with the weights
        (that ring kicks off ~3us before the sync ring), so the first proj
        matmul can start ~6us earlier; blocks 1-3 stream on the sync ring.
  proj: per 2-block group {b0,b1}: all b0 wqk chunk matmuls back-to-back
        (DMA-paced), then per chunk [b1 wqk matmul + a col-tiled wv pair]:
        the wv|wv fused stationary puts b0's v in psB partitions 0:64 and
        b1's in 64:128 concurrently (two 64-col matmuls in different PE
        column groups).
  qk:   psA (q rows 0:64, k rows 64:128) copied to SBUF with 3 casts:
        qk2a = aligned [128,512] copy, qk2b = the half-swapped duplicate.
        QK chunk parity alternates PE row halves: even chunks contract on
        partitions 64:128 (k from qk2a, q from qk2b), odd on 0:64.
  v:    one [128,512] cast psB->vtmp per group, then PE transposes per
        128-token chunk (ident halves match vtmp halves); ones column
        appended for the softmax denominator (65th PV output row).
  QK:   scores^T[s,q] per 128-wide s-chunk; causally trimmed moving width
        for the second diagonal pair of each block.
  exp:  one ACT op per chunk-pair [128,1024] -> pT fp16 with the 1/sqrt(C)
        scale via ACT's free affine pre-scale; the second diagonal pair
        uses two narrower ACT ops over exactly the causally-written cols.
  mask: triangular 128x128 multiply on diagonal chunks.
  PV:   out_aug^T[65, q] += v_aug-stationary @ pT-moving (causal widths).
        Final block drains psO stripes as each stripe's last PV lands;
        small keep-warm dummy matmuls stop the HAM clock gate from
        re-throttling the PE during the ACT-bound tail.
  out:  fp16 outT [65, T]; host divides rows 0:64 by row 64 and transposes.

fp16 everywhere on the PE (full rate, half DMA); accumulation in fp32 PSUM.
fp16 warm-up matmuls bridge the initial DMA wait so the HAM clock gate is at
K=8/8 when real work arrives.
"""

import numpy as np

import concourse.bass as bass
import concourse.mybir as mybir
import concourse.tile as tile
from concourse import bacc
from concourse.bass_utils import run_bass_kernel_spmd

B, T, C, H = 8, 2048, 1024, 64
TB = 512                 # q-block width
NBLK = T // TB           # 4 q-blocks
NC = C // 128            # 8 contraction chunks
NSC = T // 128           # 16 s-chunks
HA = H + 1               # v augmented with ones column
F32 = mybir.dt.float32
F16 = mybir.dt.float16

_compiled = {}


def build_nc():
    nc = bacc.Bacc("TRN2", target_bir_lowering=False, debug=False, num_devices=8)

    xT_d = nc.dram_tensor("xT", [C, T], F16, kind="ExternalInput").ap()
    wqk_d = nc.dram_tensor("wqk", [C, 128], F16, kind="ExternalInput").ap()
    wv_d = nc.dram_tensor("wv", [C, 128], F16, kind="ExternalInput").ap()
    # col 0:128 = causal upper-triangle mask, col 128 = ones
    tri_d = nc.dram_tensor("tri", [128, 129], F16, kind="ExternalInput").ap()
    outT_d = nc.dram_tensor("outT", [HA, T], F16, kind="ExternalOutput").ap()

    xT_r = xT_d.rearrange("(co ci) t -> ci co t", ci=128)
    wqk_r = wqk_d.rearrange("(co ci) m -> ci co m", ci=128)
    wv_r = wv_d.rearrange("(co ci) m -> ci co m", ci=128)

    with tile.TileContext(nc) as tc:
        with (
            tc.tile_pool(name="const", bufs=1) as cpool,
            tc.tile_pool(name="persist", bufs=1) as ppool,
            tc.tile_pool(name="xin", bufs=10) as xpool,
            tc.tile_pool(name="ptile", bufs=6) as pt_pool,
            tc.tile_pool(name="vtmp", bufs=2) as vtmp_pool,
            tc.tile_pool(name="outsb", bufs=2) as out_pool,
            tc.tile_pool(name="psA", bufs=2, space="PSUM") as psA_pool,
            tc.tile_pool(name="psB", bufs=1, space="PSUM") as psB_pool,
            tc.tile_pool(name="psQK", bufs=2, space="PSUM") as psQK_pool,
            tc.tile_pool(name="psO", bufs=1, space="PSUM") as psO_pool,
        ):
            wqk_s = cpool.tile([128, NC, 128], F16)
            wv_s = cpool.tile([128, NC, 128], F16)
            tri_full = cpool.tile([128, 129], F16)
            tri_s = tri_full[:, 0:128]
            ones_s = tri_full[:, 128:129]
            ident = cpool.tile([128, 64], F16)
            warm = cpool.tile([128, 512], F16)

            # weights as few large transfers on the scalar HWDGE ring (each
            # dma_start costs ~0.5us of serialized descriptor generation, and
            # the scalar queue must stay clear for the exps); chunk 0 first so
            # the first matmul's weights land ASAP
            nc.scalar.dma_start(wqk_s[:, 0:1, :], wqk_r[:, 0:1, :])
            nc.scalar.dma_start(wqk_s[:, 1:NC, :], wqk_r[:, 1:NC, :])
            nc.scalar.dma_start(wv_s[:], wv_r[:])
            nc.scalar.dma_start(tri_full[:], tri_d[:])
            # x streams in half-block transfers (each dma_start costs ~0.8us
            # of serialized sequencer descriptor generation, so chunked DMAs
            # would cap the stream at ~155 GB/s).  Block 0 rides the scalar
            # ring, which reliably starts ~3us before the sync ring and gates
            # the whole exp stream; blocks 1-3 stream on the sync ring in
            # parallel.
            x_c = [[None] * NC for _ in range(NBLK)]
            for i in range(NBLK):
                q0 = i * TB
                ring = nc.sync
                for h in range(2):
                    xh = xpool.tile([128, NC // 2, TB], F16)
                    ring.dma_start(
                        xh[:], xT_r[:, h * (NC // 2) : (h + 1) * (NC // 2),
                                    q0 : q0 + TB]
                    )
                    for c4 in range(NC // 2):
                        x_c[i][h * (NC // 2) + c4] = xh[:, c4, :]

            # PE warm-up: fp16 matmuls with no DMA deps bridge the initial
            # DMA wait and push the HAM clock gate toward K=8/8
            nc.gpsimd.memset(warm[:], 0.0)
            for w in range(8):
                ps_warm = psQK_pool.tile([128, 512], F32, tag="psQK")
                nc.tensor.matmul(ps_warm[:], warm[:, 0:128], warm[:],
                                 start=True, stop=True)
            for h2 in range(2):
                sl = ident[h2 * 64 : (h2 + 1) * 64, :]
                nc.gpsimd.memset(sl, 0.0)
                nc.gpsimd.affine_select(
                    out=sl,
                    in_=sl,
                    compare_op=mybir.AluOpType.not_equal,
                    fill=1.0,
                    base=0,
                    pattern=[[-1, 64]],
                    channel_multiplier=1,
                )

            # qk2a: q rows 0:64, k rows 64:128 (aligned copy of psA)
            # qk2b: k rows 0:64, q rows 64:128 (half-swapped duplicate)
            qk2a = ppool.tile([128, T], F16)
            qk2b = ppool.tile([128, T], F16)
            v_s = ppool.tile([128, NSC * HA], F16)

            psA_t = [None] * NBLK
            psB_t = [None] * (NBLK // 2)
            vtmp_t = [None] * (NBLK // 2)

            def wqk_part(b, c0, c1):
                """q/k proj matmuls for block b, chunks [c0, c1); allocates
                psA at c0==0 and emits the casts at c1==NC."""
                if c0 == 0:
                    psA_t[b] = psA_pool.tile([128, TB], F32, name="psA")
                psA = psA_t[b]
                for c in range(c0, c1):
                    nc.tensor.matmul(
                        psA[:], wqk_s[:, c, :], x_c[b][c][:],
                        start=(c == 0), stop=(c == NC - 1),
                    )
                if c1 == NC:
                    qk_casts(b)
                    if b < 2:
                        # PE idles for the cast latency before the first QKs;
                        # dangling LDWEIGHTS keep the HAM clock gate from
                        # re-throttling during the ramp
                        for _ in range(10):
                            nc.tensor.ldweights(warm[:, 0:128])

            def wv_part(gi, c0, c1):
                """col-tiled wv pairs: b0 -> psB[0:64] (PE col group 0-1),
                b1 -> psB[64:128] (col group 2-3) run concurrently."""
                b0, b1 = 2 * gi, 2 * gi + 1
                if c0 == 0:
                    psB_t[gi] = psB_pool.tile([128, TB], F32, name="psB")
                psB = psB_t[gi]
                for c in range(c0, c1):
                    nc.tensor.matmul(
                        psB[0:64], wv_s[:, c, 0:64], x_c[b0][c][:],
                        start=(c == 0), stop=(c == NC - 1),
                    )
                    nc.tensor.matmul(
                        psB[64:128], wv_s[:, c, 64:128], x_c[b1][c][:],
                        start=(c == 0), stop=(c == NC - 1),
                    )

            def qk_casts(i):
                q0 = i * TB
                psA = psA_t[i]
                nc.vector.tensor_copy(qk2a[:, q0 : q0 + TB], psA[0:128, :])
                if i < 2:
                    # scalar engine is idle before the exp stream starts:
                    # run the q-dup cast there in parallel with the DVE pair
                    nc.scalar.copy(qk2b[64:128, q0 : q0 + TB], psA[0:64, :])
                else:
                    nc.vector.tensor_copy(qk2b[64:128, q0 : q0 + TB], psA[0:64, :])
                nc.vector.tensor_copy(qk2b[0:64, q0 : q0 + TB], psA[64:128, :])

            def vt_part(gi, db):
                """PE transposes for one block of the group; db==0 also does
                the psB->vtmp cast covering both blocks."""
                b0 = 2 * gi
                if db == 0:
                    vtmp_t[gi] = vtmp_pool.tile([128, TB], F16, name="vtmp")
                    nc.vector.tensor_copy(vtmp_t[gi][:], psB_t[gi][:])
                vtmp = vtmp_t[gi]
                r = slice(db * 64, db * 64 + 64)
                for j4 in range(TB // 128):
                    sj = (TB // 128) * (b0 + db) + j4
                    ps_vt = psQK_pool.tile([128, 64], F16, tag="psQK")
                    nc.tensor.transpose(
                        ps_vt[:],
                        vtmp[r, j4 * 128 : (j4 + 1) * 128],
                        ident[r, :],
                    )
                    nc.vector.tensor_copy(v_s[:, sj * HA : sj * HA + H], ps_vt[:])
                    # ones column via gpsimd (idle engine) to keep DVE free
                    nc.gpsimd.memset(
                        v_s[:, sj * HA + H : sj * HA + HA], 1.0
                    )

            def qk_mm(j, q0, lo, dst):
                """scores^T for s-chunk j against q cols [q0+lo, q0+TB);
                chunk parity picks the PE row half."""
                if j % 2 == 0:
                    stat, mov = qk2a[64:128, :], qk2b[64:128, :]
                else:
                    stat, mov = qk2b[0:64, :], qk2a[0:64, :]
                nc.tensor.matmul(
                    dst,
                    stat[:, j * 128 : (j + 1) * 128],
                    mov[:, q0 + lo : q0 + TB],
                    start=True, stop=True,
                )

            scale = float(1.0 / np.sqrt(C))

            # pair list in stream order: (block, pair-in-block)
            pairs = [(i, g) for i in range(NBLK)
                     for g in range((TB // 128) * (i + 1) // 2)]
            pT_t = {}
            psO_t = [None] * NBLK
            out_sb_last = [None]

            def pair_geom(i, g):
                q0 = i * TB
                js = [2 * g, 2 * g + 1]
                ds = [j * 128 - q0 for j in js]
                los = [max(dd, 0) for dd in ds]
                diagB = los[0] > 0  # second diagonal pair of the block
                return q0, js, ds, los, diagB

            def scores(i, g):
                """QK + exp (+nothing that needs v): produces pT_t[(i,g)]."""
                q0, js, ds, los, diagB = pair_geom(i, g)
                psQK = psQK_pool.tile([128, 1024], F32, tag="psQK")
                for h2 in range(2):
                    lo = los[h2] if diagB else 0
                    qk_mm(js[h2], q0, lo,
                          psQK[:, h2 * 512 + lo : (h2 + 1) * 512])
                pT = pt_pool.tile([128, 1024], F16)
                pT_t[(i, g)] = pT
                if diagB:
                    # two ACT ops over exactly the causally-written cols
                    for h2 in range(2):
                        lo = los[h2]
                        nc.scalar.activation(
                            pT[:, h2 * 512 + lo : (h2 + 1) * 512],
                            psQK[:, h2 * 512 + lo : (h2 + 1) * 512],
                            mybir.ActivationFunctionType.Exp,
                            scale=scale,
                        )
                else:
                    nc.scalar.activation(
                        pT[:], psQK[:], mybir.ActivationFunctionType.Exp,
                        scale=scale,
                    )
                for h2 in range(2):
                    d = ds[h2]
                    if d >= 0:  # diagonal chunk: triangular causal mask
                        pj = pT[:, h2 * 512 : (h2 + 1) * 512]
                        nc.vector.tensor_mul(
                            pj[:, d : d + 128], pj[:, d : d + 128], tri_s[:]
                        )

            def pv(i, g):
                q0, js, ds, los, _ = pair_geom(i, g)
                nsc_i = (TB // 128) * (i + 1)
                if g == 0:
                    psO_t[i] = psO_pool.tile([HA, TB], F32, name="psO")
                    if i == NBLK - 1:
                        out_sb_last[0] = out_pool.tile([HA, TB], F16,
                                                       name="out_sb_last")
                psO = psO_t[i]
                pT = pT_t.pop((i, g))
                if i >= 2:
                    # keep-warm: the late blocks are ACT-bound and PE
                    # micro-idles re-throttle the HAM clock gate; dangling
                    # LDWEIGHTS fill the exp-wait stall and are overwritten
                    # by the next matmul's own weight load
                    nc.tensor.ldweights(warm[:, 0:128])
                    nc.tensor.ldweights(warm[:, 0:128])
                for h2 in range(2):
                    j, lo = js[h2], los[h2]
                    pj = pT[:, h2 * 512 : (h2 + 1) * 512]
                    nc.tensor.matmul(
                        psO[:, lo:TB],
                        v_s[:, j * HA : (j + 1) * HA],
                        pj[:, lo:TB],
                        start=(j == 0), stop=(j == nsc_i - 1),
                    )
                    if i == NBLK - 1 and j >= nsc_i - 4:
                        # drain each psO stripe as its last PV lands
                        p = j - (nsc_i - 4)
                        sl = slice(p * 128, (p + 1) * 128)
                        nc.vector.tensor_copy(out_sb_last[0][:, sl], psO[:, sl])
                        nc.gpsimd.dma_start(
                            outT_d[:, q0 + p * 128 : q0 + (p + 1) * 128],
                            out_sb_last[0][:, sl],
                        )
                if i < NBLK - 1 and g == nsc_i // 2 - 1:
                    out_sb = out_pool.tile([HA, TB], F16)
                    nc.vector.tensor_copy(out_sb[:], psO[:])
                    nc.gpsimd.dma_start(outT_d[:, q0 : q0 + TB], out_sb[:])

            # Hand-scheduled emission: the Tile static scheduler re-sorts by
            # its own cost model, so each group is pinned to an increasing
            # tile_wait_until slot (a pure scheduler-time gate, no runtime
            # waits) to force the engine-queue order.  Bulk proj/v work is
            # sliced between scores pairs so the exp stream stays dense; the
            # scores stream runs ahead of the pv stream by at most 6 pairs
            # (the pT pool depth).
            S = lambda k: scores(*pairs[k])
            P = lambda k: pv(*pairs[k])
            groups = [
                lambda: wqk_part(0, 0, NC),     # block 0 proj + casts
                lambda: wqk_part(1, 0, 4),      # fills the casts(0) latency
                lambda: (S(0), S(1)),           # attn0 scores
                lambda: wqk_part(1, 4, NC),     # block 1 proj + casts
                lambda: S(2),                   # (1,0)
                lambda: wv_part(0, 0, 4),
                lambda: S(3),                   # (1,1)
                lambda: wv_part(0, 4, NC),
                lambda: S(4),                   # (1,2)
                lambda: vt_part(0, 0),
                lambda: S(5),                   # (1,3)
                lambda: vt_part(0, 1),
                lambda: (P(0), P(1)),           # attn0 pv
                lambda: wqk_part(2, 0, NC),     # block 2 proj + casts
                lambda: P(2),                   # (1,0)
                lambda: S(6),                   # (2,0)
                lambda: P(3),
                lambda: wqk_part(3, 0, 4),
                lambda: S(7),                   # (2,1)
                lambda: wqk_part(3, 4, NC),     # block 3 proj + casts
                lambda: P(4),
                lambda: S(8),                   # (2,2)
                lambda: wv_part(1, 0, 4),
                lambda: P(5),
                lambda: S(9),                   # (2,3)
                lambda: wv_part(1, 4, NC),
                lambda: P(6),                   # (2,0)
                lambda: vt_part(1, 0),
                lambda: S(10),                  # (2,4)
                lambda: P(7),
                lambda: vt_part(1, 1),
                lambda: S(11),                  # (2,5)
                lambda: P(8),
                lambda: S(12),                  # (3,0)
                lambda: P(9),
                lambda: S(13),
                lambda: P(10),
                lambda: S(14),
                lambda: P(11),
                lambda: S(15),
                lambda: P(12),
                lambda: S(16),
                lambda: P(13),
                lambda: S(17),
                lambda: P(14),
                lambda: S(18),
                lambda: P(15),
                lambda: S(19),
                lambda: (P(16), P(17), P(18), P(19)),
            ]
            for group in groups:
                group()

    nc.compile()
    return nc


def _get_nc():
    if "nc" not in _compiled:
        _compiled["nc"] = build_nc()
    return _compiled["nc"]


def make_in_maps(x, Wk, Wq, Wv):
    x = np.asarray(x, dtype=np.float32)
    Wk = np.asarray(Wk, dtype=np.float32)
    Wq = np.asarray(Wq, dtype=np.float32)
    Wv = np.asarray(Wv, dtype=np.float32)
    # raw Wq (no 1/sqrt(C) here — that scale rides the exp's affine pre-scale)
    wqk = np.concatenate([Wq, Wk], axis=1).astype(np.float16)  # [C, 128]
    wvd = np.concatenate([Wv, Wv], axis=1).astype(np.float16)  # [C, 128] dup
    tri = np.ones((128, 129), dtype=np.float16)
    tri[:, 0:128] = np.triu(np.ones((128, 128), dtype=np.float16))
    in_maps = []
    for b in range(B):
        in_maps.append(
            {
                "xT": np.ascontiguousarray(x[b].T.astype(np.float16)),
                "wqk": wqk,
                "wv": wvd,
                "tri": tri,
            }
        )
    return in_maps


def postprocess(results):
    outs = []
    for b in range(B):
        outT = results[b]["outT"].astype(np.float32)  # [65, T]
        out = (outT[:H] / outT[H : H + 1]).T  # [T, H]
        outs.append(out)
    return np.stack(outs).astype(np.float32)


def run(x, Wk, Wq, Wv, trace=False, **kw):
    nc = _get_nc()
    in_maps = make_in_maps(x, Wk, Wq, Wv)
    res = run_bass_kernel_spmd(
        nc, in_maps, core_ids=list(range(B)), trace=trace, **kw
    )
    return postprocess(res.results), res


def kernel(x, Wk, Wq, Wv):
    out, _ = run(x, Wk, Wq, Wv, trace=False)
    return out
